# revision 12
# baseline (speedup 1.0000x reference)
"""KANLinear forward on 8 Trainium2 NeuronCores (data-parallel over batch).

Factorization
-------------
reference computes, per token row x (after nan/clip/renorm preprocessing):
    y = silu(x) @ base_weight.T + einsum('big,oig->bo', bsplines(x), sw*scaler)

The cubic B-spline bases over the uniform grid (h=0.4, knots -2.2..2.2) are
    B_g(x) = N3(s - g),  s = 2.5*x + 5.5,  g = 0..7
with N3 the cardinal cubic B-spline on [0,4].  Using the symmetric identity
    6*N3(t) = relu(min(t, 4-t))^3 - 4*relu(min(t, 4-t) - 1)^3
both cube terms are computed on-device by two fused custom-DVE instructions
(8 pipeline stages each) that evaluate all 8 shifts per element in one pass
via PageIdx paging.  The spline einsum and the silu base path then collapse
into a single K=4608 bf16 matmul per 128-row output tile:
    K rows [g*512 + i] = (sw[o,i,g]*scaler[o,i])/6 ;  K rows [4096+i] = bw[o,i]
Batch dim (16384) is sharded 2048 rows/core; weights are replicated.
"""

import numpy as np

B, IN_F, OUT_F, NG = 16384, 512, 512, 8
N_CORES = 8
BPC = B // N_CORES            # batch rows per core
BS = 512                      # batch-column slice processed per step
N_BS = BPC // BS              # 4 slices
N_IT = IN_F // 128            # 4 input-feature partition tiles
KC = NG * N_IT + N_IT         # 36 K-chunks of 128
GAMMA = float(4.0 ** (1.0 / 3.0))

_ops = {}
_kernel_cache = {}


def _register_ops():
    """Register the three custom DVE ops (idempotent)."""
    if _ops:
        return _ops
    import concourse.dve_ops as dve_ops
    from concourse.dve_spec import (
        Spec, Src0, Src1, C0, C1, C2, One, PageIdx, relu, sq, maxx, minn, lower,
    )
    from concourse.dve_uop import DveOpSpec

    def page_idx_np(in0, s0, s1):
        S = in0.shape[1]
        return (s0 + s1 * np.arange(S, dtype=np.float64)).astype(np.float32)[
            None, :, None
        ]

    def pre_ref(in0, in1, s0, s1, imm2):
        t = np.minimum(np.maximum(in0, np.float32(s0)), np.float32(s1))
        t = ((t + np.float32(1)) - np.float32(1)).astype(np.float32)
        return (t * np.float32(imm2)).astype(np.float32)

    def z_ref(in0, in1, s0, s1, imm2):
        t = (in0 + page_idx_np(in0, s0, s1)).astype(np.float32)
        m = np.minimum(t, np.float32(imm2) - t)
        zp = np.maximum(m + np.float32(s1), np.float32(0))
        return (zp * zp * zp).astype(np.float32)

    def w_ref(in0, in1, s0, s1, imm2):
        t = (in0 + page_idx_np(in0, s0, s1)).astype(np.float32)
        m = np.minimum(t, np.float32(4.0) - t)
        wp = np.maximum(m, np.float32(0))
        return (wp * wp * wp - in1).astype(np.float32)

    # KAN_PRE: out = imm2 * ((clip(x, s0, s1) + 1) - 1)
    pre_spec = Spec(
        body=((minn(maxx(Src0, C0), C1) + One) - One) * C2,
        reference=pre_ref,
    )
    # KAN_Z (paged, single-src): t = x + (s0 + page*s1); m = min(t, imm2 - t)
    #   out = relu(m + s1)^3     [call with s0=5.5*G, s1=-G, imm2=4*G  ->  4*z+^3]
    _pgz = PageIdx(C0, C1)
    _tz = Src0 + _pgz
    _mz = minn(_tz, C2 - _tz)
    _zp = relu(_mz + C1)
    z_spec = Spec(body=sq(_zp) * _zp, reference=z_ref)
    # KAN_W (paged, two-src): t = x + (s0 + page*s1); m = min(t, imm2 - t)
    #   out = relu(m)^3 - in1    [call with s0=5.5, s1=-1, imm2=4 -> 6*N3(s-g)]
    # in1 is passed flat [P, S*N] so the encoding stays TTSS (imm2 usable).
    _pgw = PageIdx(C0, C1)
    _tw = Src0 + _pgw
    _mw = minn(_tw, C2 - _tw)
    _wp = relu(_mw)
    w_spec = Spec(body=sq(_wp) * _wp - Src1, reference=w_ref)

    for name, spec, subdim in (
        ("KAN_PRE", pre_spec, False),
        ("KAN_Z", z_spec, True),
        ("KAN_W", w_spec, True),
    ):
        if name in dve_ops._SUB_OPCODE_FOR_NAME:
            _ops[name] = next(o for o in dve_ops.OPS if o.name == name)
            continue
        row = dve_ops._CUSTOM_DVE_ROW_BASE + len(dve_ops.OPS)
        assert row < 0x20, "custom-DVE row overflow"
        shas = {}
        for ver in ("v3", "v4"):
            try:
                tmp = DveOpSpec(
                    name=name,
                    opcode=row,
                    uops=lower(spec, ver=ver),
                    rd1_en=dve_ops.has_src1(spec),
                )
                shas[ver] = tmp.sha(ver)
            except Exception:
                pass
        op = dve_ops.DveOp(name, spec, subdim=subdim, uops_sha=shas)
        dve_ops.OPS.append(op)
        dve_ops._SUB_OPCODE_FOR_NAME[name] = row
        dve_ops.CUSTOM_DVE_SPECS[name] = spec
        _ops[name] = op
    return _ops


def _build_kernel():
    """Build the Bass/Tile kernel (same program for every core)."""
    if "nc" in _kernel_cache:
        return _kernel_cache["nc"]
    import concourse.bacc as bacc
    import concourse.mybir as mybir
    import concourse.tile as tile
    from concourse.bass import ts

    ops = _register_ops()
    f32 = mybir.dt.float32
    bf16 = mybir.dt.bfloat16

    nc = bacc.Bacc()
    xT = nc.dram_tensor("xT", [IN_F, BPC], f32, kind="ExternalInput")
    V = nc.dram_tensor("V", [KC * 128, OUT_F], bf16, kind="ExternalInput")
    yT = nc.dram_tensor("yT", [OUT_F, BPC], f32, kind="ExternalOutput")

    with tile.TileContext(nc) as tc:
        with (
            tc.tile_pool(name="vpool", bufs=1) as vpool,
            tc.tile_pool(name="xin", bufs=3) as xin_pool,
            tc.tile_pool(name="xs", bufs=3) as xs_pool,
            tc.tile_pool(name="xs2", bufs=3) as xs2_pool,
            tc.tile_pool(name="z3", bufs=2) as z3_pool,
            tc.tile_pool(name="feat", bufs=8) as feat_pool,
            tc.tile_pool(name="silu", bufs=8) as silu_pool,
            tc.tile_pool(name="ysb", bufs=4) as ysb_pool,
            tc.tile_pool(name="psum", bufs=8, space="PSUM") as psum_pool,
        ):
            # Weights: one SBUF tile [128, KC, OUT_F]; partition = K within chunk.
            # Chunk c = it*9 + g (g=8 is the silu row block) so the first DMA
            # quarter covers everything i-tile 0 needs.
            v_sb = vpool.tile([128, KC, OUT_F], bf16)
            v_view = V[:].rearrange("(kc p) o -> p kc o", p=128)
            for q in range(4):
                nc.sync.dma_start(
                    v_sb[:, ts(q, KC // 4), :], v_view[:, ts(q, KC // 4), :]
                )

            for bs in range(N_BS):
                accs = [
                    psum_pool.tile([128, BS], f32, name=f"acc{o}", tag="acc")
                    for o in range(N_IT)
                ]
                for it in range(N_IT):
                    xin = xin_pool.tile([128, BS], f32)
                    nc.sync.dma_start(xin[:], xT[ts(it, 128), ts(bs, BS)])
                    xs = xs_pool.tile([128, BS], f32)
                    nc.vector._custom_dve(
                        ops["KAN_PRE"], out=xs[:], in0=xin[:],
                        s0=-1.1, s1=1.1, imm2=2.5,
                    )
                    xs2 = xs2_pool.tile([128, BS], f32)
                    nc.scalar.activation(
                        xs2[:], xs[:], mybir.ActivationFunctionType.Copy,
                        scale=GAMMA,
                    )
                    sil = silu_pool.tile([128, BS], bf16)
                    nc.scalar.activation(
                        sil[:], xs[:], mybir.ActivationFunctionType.Silu,
                        scale=0.4,
                    )
                    z3 = z3_pool.tile([128, NG, BS], f32)
                    nc.vector._custom_dve(
                        ops["KAN_Z"],
                        out=z3[:],
                        in0=xs2[:].unsqueeze(1).broadcast_to([128, NG, BS]),
                        s0=5.5 * GAMMA, s1=-GAMMA, imm2=4.0 * GAMMA,
                    )
                    ft = feat_pool.tile([128, NG, BS], bf16)
                    nc.vector._custom_dve(
                        ops["KAN_W"],
                        out=ft[:],
                        in0=xs[:].unsqueeze(1).broadcast_to([128, NG, BS]),
                        in1=z3[:].rearrange("p s n -> p (s n)"),
                        s0=5.5, s1=-1.0, imm2=4.0,
                    )
                    # Drain this i-tile's K-chunks into all 4 output tiles
                    # right away so PE trails the DVE by one i-tile only.
                    for o in range(N_IT):
                        for g in range(NG):
                            nc.tensor.matmul(
                                accs[o][:],
                                v_sb[:, it * (NG + 1) + g, ts(o, 128)],
                                ft[:, g, :],
                                start=(it == 0 and g == 0),
                                stop=False,
                            )
                        nc.tensor.matmul(
                            accs[o][:],
                            v_sb[:, it * (NG + 1) + NG, ts(o, 128)],
                            sil[:],
                            start=False,
                            stop=(it == N_IT - 1),
                        )
                for o in range(N_IT):
                    ysb = ysb_pool.tile([128, BS], f32)
                    nc.scalar.copy(ysb[:], accs[o][:])
                    nc.sync.dma_start(yT[ts(o, 128), ts(bs, BS)], ysb[:])

    nc.compile()
    _kernel_cache["nc"] = nc
    return nc


def _build_V(base_weight, spline_weight, spline_scaler):
    # K-chunk c = it*9 + g: rows c*128+p.  g<8 -> sw[o, it*128+p, g]*sc/6,
    # g=8 -> base_weight[o, it*128+p].
    sw = spline_weight.astype(np.float32) * spline_scaler.astype(np.float32)[:, :, None]
    vs = np.transpose(sw, (2, 1, 0)) / np.float32(6.0)  # [g, i, o]
    bwT = base_weight.astype(np.float32).T  # [i, o]
    V = np.empty((KC * 128, OUT_F), dtype=np.float32)
    for it in range(N_IT):
        isl = slice(it * 128, (it + 1) * 128)
        for g in range(NG):
            c = it * (NG + 1) + g
            V[c * 128 : (c + 1) * 128] = vs[g, isl, :]
        c = it * (NG + 1) + NG
        V[c * 128 : (c + 1) * 128] = bwT[isl, :]
    import ml_dtypes
    return np.ascontiguousarray(V.astype(ml_dtypes.bfloat16))


def kernel(x, base_weight, spline_weight, spline_scaler, grid):
    from concourse.bass_utils import run_bass_kernel_spmd

    nc = _build_kernel()
    Vb = _build_V(base_weight, spline_weight, spline_scaler)
    x = np.asarray(x, dtype=np.float32)
    in_maps = []
    for c in range(N_CORES):
        xTc = np.ascontiguousarray(x[c * BPC : (c + 1) * BPC, :].T)
        in_maps.append({"xT": xTc, "V": Vb})
    res = run_bass_kernel_spmd(nc, in_maps, core_ids=list(range(N_CORES)))
    y = np.empty((B, OUT_F), dtype=np.float32)
    for c in range(N_CORES):
        y[c * BPC : (c + 1) * BPC, :] = res.results[c]["yT"].T
    return y
